# revision 5
# baseline (speedup 1.0000x reference)
"""Trainium2 Bass kernel for a 2-layer GCN (nn_CorrelationGNN).

Math (reference):
    src,dst = edges + self loops;  deg over dst;  dinv = deg^-1/2
    h1 = relu(S @ (x @ W0) + b0),  S = D^-1/2 (A+I) D^-1/2
    h2 = relu(S @ (h1 @ W1) + b1)
    out = h2 @ Wf + bf

Factorization used: S @ (h W) = dinv * Agg(dinv * h) @ W, where Agg is the
pure 0/1 adjacency gather-sum (S commutes with the feature matmul).

Distribution: destination nodes sharded across 8 cores (12500/core, padded
to 12544 = 128*98).  Ranks are degree-sorted; rank r -> (p=r%128, g=r//128),
table row within a core slice = p*98+g.  Gather source is an fp16 table
[100352, 128] (row = 32 feats + 96 zeros = 256B) assembled per core from an
AllGather of compact fp16 slices.  Edges are gathered with gpsimd dma_gather
(int16 idxs, 4 SWDGE queues, <=1024 idxs/inst) as 4 source-quarter streams;
per (quarter, g-column) the slot count K is the max over all cores so the
traced program is identical on every core (SPMD).
"""

import numpy as np

import concourse.bass as bass  # noqa: F401
import concourse.bacc as bacc
import concourse.mybir as mybir
from concourse.tile import TileContext
from concourse.bass_utils import run_bass_kernel_spmd

P = 128
N = 100000
F = 32
NPC = 12500          # real nodes per core
G = 98               # g-columns per core
NPCP = P * G         # padded nodes per core = 12544
NROWS = 8 * NPCP     # global table rows = 100352
QROWS = NROWS // 4   # 25088, int16-addressable quarter
QZREL = 12543        # guaranteed-zero pad row, same offset in every quarter
KCAP = 8             # slots per dma_gather inst (8*128 = 1024 idx cap)
FDT = mybir.dt.float32
HDT = mybir.dt.float16


def _build_plan_and_offsets(edge_index):
    src = np.asarray(edge_index[0], dtype=np.int64)
    dst = np.asarray(edge_index[1], dtype=np.int64)
    loops = np.arange(N, dtype=np.int64)
    src = np.concatenate([src, loops])
    dst = np.concatenate([dst, loops])

    deg = np.bincount(dst, minlength=N).astype(np.float64)
    dinv = (1.0 / np.sqrt(deg)).astype(np.float32)

    node_core = np.arange(N) // NPC
    rank = np.empty(N, dtype=np.int64)
    perms = []
    for c in range(8):
        nodes = np.arange(c * NPC, (c + 1) * NPC)
        order = np.argsort(-deg[nodes], kind="stable")
        perm = nodes[order]
        perms.append(perm)
        rank[perm] = np.arange(NPC)
    trow = node_core * NPCP + (rank % P) * G + (rank // P)
    quarter = trow // QROWS
    qrel = (trow % QROWS).astype(np.int32)

    # per-core edges sorted by (dst rank, src quarter)
    edges = []
    cnt_rq = np.zeros((8, NPC * 4), dtype=np.int32)
    for c in range(8):
        m = (dst // NPC) == c
        s, d = src[m], dst[m]
        key = rank[d] * 4 + quarter[s]
        order = np.argsort(key, kind="stable")
        edges.append((qrel[s][order], key[order]))
        cnt_rq[c] = np.bincount(key, minlength=NPC * 4)

    crq = cnt_rq.reshape(8, NPC, 4)
    K = np.zeros((G, 4), dtype=np.int32)
    for g in range(G):
        K[g] = crq[:, g * P : (g + 1) * P, :].max(axis=(0, 1))

    # shared instruction plan: (q, g, k0, kc, col0)
    plan = []
    col = 0
    for q in range(4):
        for g in range(G):
            k0 = 0
            while k0 < int(K[g, q]):
                kc = min(KCAP, int(K[g, q]) - k0)
                plan.append((q, g, k0, kc, col))
                col += kc * P // 16
                k0 += kc
    totc = col

    # per-core offset tables [16, totc], idx i of inst at [i%16, col0+i//16]
    offs_all = []
    for c in range(8):
        qr, key = edges[c]
        cnt = np.bincount(key, minlength=NPC * 4)
        ptr = np.zeros(NPC * 4 + 1, dtype=np.int64)
        np.cumsum(cnt, out=ptr[1:])
        offs = np.full((totc * 16,), QZREL, dtype=np.int16)
        for (q, g, k0, kc, col0) in plan:
            base = col0 * 16
            lo = g * P
            nreal = min(P, NPC - lo)
            # vectorized over p
            ps = np.arange(nreal)
            rk = lo + ps
            a = ptr[rk * 4 + q]
            b = ptr[rk * 4 + q + 1]
            for kk in range(kc):
                k = k0 + kk
                sel = (a + k) < b
                pos = base + kk * P + ps[sel]
                offs[pos] = qr[(a + k)[sel]]
        offs_all.append(offs.reshape(totc, 16).T.copy())

    return plan, totc, dinv, perms, offs_all


def _build_program(plan, totc):
    nc = bacc.Bacc(
        "TRN2", target_bir_lowering=False, debug=False, num_devices=8,
        num_swdge_queues=4,
    )
    x_own = nc.declare_dram_parameter("x_own", [P, G * F], FDT, isOutput=False)
    dinv_own = nc.declare_dram_parameter("dinv_own", [P, G], FDT, isOutput=False)
    offs = nc.declare_dram_parameter("offs", [P, totc], mybir.dt.int16, isOutput=False)
    W0 = nc.declare_dram_parameter("W0", [F, F], FDT, isOutput=False)
    W1 = nc.declare_dram_parameter("W1", [F, F], FDT, isOutput=False)
    Wf = nc.declare_dram_parameter("Wf", [F, F], FDT, isOutput=False)
    b0 = nc.declare_dram_parameter("b0", [F, 1], FDT, isOutput=False)
    b1 = nc.declare_dram_parameter("b1", [F, 1], FDT, isOutput=False)
    bf = nc.declare_dram_parameter("bf", [F, 1], FDT, isOutput=False)
    out_own = nc.declare_dram_parameter("out_own", [P, G * F], FDT, isOutput=True)

    cc_in = nc.dram_tensor("cc_in", [NPCP, F], HDT)
    cc_out = nc.dram_tensor("cc_out", [NROWS, F], HDT, addr_space="Shared")
    table = nc.dram_tensor("table", [NROWS, 4 * F], HDT)

    from concourse.masks import make_identity

    with TileContext(nc) as tc:
        with (
            tc.tile_pool(name="persist", bufs=1) as pp,
            tc.tile_pool(name="gpool", bufs=6) as gp,
            tc.tile_pool(name="spool", bufs=4) as sp,
            tc.tile_pool(name="psum", bufs=2, space="PSUM") as psp,
        ):
            offs_t = pp.tile([P, totc], mybir.dt.int16)
            nc.sync.dma_start(out=offs_t[:], in_=offs[:])
            dinv_t = pp.tile([P, G], FDT)
            nc.sync.dma_start(out=dinv_t[:], in_=dinv_own[:])
            w0_t = pp.tile([F, F], FDT)
            nc.sync.dma_start(out=w0_t[:], in_=W0[:])
            w1_t = pp.tile([F, F], FDT)
            nc.sync.dma_start(out=w1_t[:], in_=W1[:])
            wf_t = pp.tile([F, F], FDT)
            nc.sync.dma_start(out=wf_t[:], in_=Wf[:])
            b0_t = pp.tile([F, 1], FDT)
            nc.sync.dma_start(out=b0_t[:], in_=b0[:])
            b1_t = pp.tile([F, 1], FDT)
            nc.sync.dma_start(out=b1_t[:], in_=b1[:])
            bf_t = pp.tile([F, 1], FDT)
            nc.sync.dma_start(out=bf_t[:], in_=bf[:])
            ident = pp.tile([P, P], FDT)
            make_identity(nc, ident[:])

            xcur = pp.tile([P, G * F], FDT, tag="xcur")
            nc.sync.dma_start(out=xcur[:], in_=x_own[:])
            agg = pp.tile([P, G * F], FDT, tag="agg")
            xd_own = pp.tile([P, G * F], HDT, tag="xdown")

            dinv_b = dinv_t[:].to_broadcast([P, G, F])

            def scale_to_table(src_tile, scope):
                with nc.named_scope(scope):
                    nc.vector.tensor_tensor(
                        out=xd_own[:].rearrange("p (g f) -> p g f", f=F),
                        in0=src_tile[:].rearrange("p (g f) -> p g f", f=F),
                        in1=dinv_b,
                        op=mybir.AluOpType.mult,
                    )
                    nc.sync.dma_start(out=cc_in[:], in_=xd_own[:])
                    nc.gpsimd.collective_compute(
                        "AllGather",
                        mybir.AluOpType.bypass,
                        replica_groups=[list(range(8))],
                        ins=[cc_in[:]],
                        outs=[cc_out[:]],
                    )
                    for qq in range(4):
                        nc.sync.dma_start(
                            out=table[qq * QROWS : (qq + 1) * QROWS, :F],
                            in_=cc_out[qq * QROWS : (qq + 1) * QROWS, :],
                        )

            def gather_layer(scope):
                with nc.named_scope(scope):
                    nc.vector.memset(agg[:], 0.0)
                    for (q, g, k0, kc, col0) in plan:
                        gt = gp.tile([P, KCAP, 4 * F], HDT, tag="g")
                        nc.gpsimd.dma_gather(
                            out_ap=gt[:, :kc, :],
                            in_ap=table[q * QROWS : (q + 1) * QROWS, :],
                            idxs_ap=offs_t[:, col0 : col0 + kc * P // 16],
                            num_idxs=kc * P,
                            num_idxs_reg=kc * P,
                            elem_size=4 * F,
                            queue_num=(q * G + g) % 4,
                        )
                        red = sp.tile([P, 4, F], FDT, tag="red")
                        k = kc
                        if k > 4:
                            h = k - 4
                            nc.vector.tensor_add(
                                out=red[:, :h, :], in0=gt[:, 0:h, :F], in1=gt[:, 4 : 4 + h, :F]
                            )
                            if h < 4:
                                nc.vector.tensor_copy(out=red[:, h:4, :], in_=gt[:, h:4, :F])
                            k = 4
                        else:
                            nc.vector.tensor_copy(out=red[:, :k, :], in_=gt[:, :k, :F])
                        while k > 1:
                            h = k // 2
                            r = k - h
                            nc.vector.tensor_add(
                                out=red[:, :h, :], in0=red[:, :h, :], in1=red[:, r : r + h, :]
                            )
                            k = r
                        nc.vector.tensor_add(
                            out=agg[:, g * F : (g + 1) * F],
                            in0=agg[:, g * F : (g + 1) * F],
                            in1=red[:, 0, :],
                        )

            def layer_tail(W_t, bias_t, relu, dest, scope, W2_t=None, bias2_t=None):
                with nc.named_scope(scope):
                    nc.vector.tensor_tensor(
                        out=agg[:].rearrange("p (g f) -> p g f", f=F),
                        in0=agg[:].rearrange("p (g f) -> p g f", f=F),
                        in1=dinv_b,
                        op=mybir.AluOpType.mult,
                    )
                    for g in range(G):
                        ps1 = psp.tile([F, P], FDT, tag="ps1")
                        nc.tensor.matmul(
                            out=ps1[:], lhsT=agg[:, g * F : (g + 1) * F], rhs=ident[:],
                            start=True, stop=True,
                        )
                        s1 = sp.tile([F, P], FDT, tag="s1")
                        nc.vector.tensor_copy(out=s1[:], in_=ps1[:])
                        ps2 = psp.tile([F, P], FDT, tag="ps2")
                        nc.tensor.matmul(out=ps2[:], lhsT=W_t[:], rhs=s1[:], start=True, stop=True)
                        s2 = sp.tile([F, P], FDT, tag="s2")
                        if relu:
                            nc.scalar.activation(
                                out=s2[:], in_=ps2[:],
                                func=mybir.ActivationFunctionType.Relu,
                                bias=b0_t[:, :1] if bias_t is b0_t else bias_t[:, :1],
                                scale=1.0,
                            )
                        else:
                            nc.vector.tensor_scalar(
                                out=s2[:], in0=ps2[:], scalar1=bias_t[:, :1],
                                scalar2=None, op0=mybir.AluOpType.add,
                            )
                        if W2_t is not None:
                            ps3 = psp.tile([F, P], FDT, tag="ps3")
                            nc.tensor.matmul(out=ps3[:], lhsT=W2_t[:], rhs=s2[:], start=True, stop=True)
                            s2b = sp.tile([F, P], FDT, tag="s2b")
                            nc.vector.tensor_scalar(
                                out=s2b[:], in0=ps3[:], scalar1=bias2_t[:, :1],
                                scalar2=None, op0=mybir.AluOpType.add,
                            )
                            s2 = s2b
                        psb = psp.tile([P, F], FDT, tag="psb")
                        nc.tensor.matmul(
                            out=psb[:], lhsT=s2[:], rhs=ident[:F, :F], start=True, stop=True
                        )
                        nc.vector.tensor_copy(out=dest[:, g * F : (g + 1) * F], in_=psb[:])

            scale_to_table(xcur, "table0")
            gather_layer("gather0")
            layer_tail(w0_t, b0_t, relu=True, dest=xcur, scope="tail0")
            scale_to_table(xcur, "table1")
            gather_layer("gather1")
            outt = pp.tile([P, G * F], FDT, tag="outt")
            layer_tail(
                w1_t, b1_t, relu=True, dest=outt, scope="tail1", W2_t=wf_t, bias2_t=bf_t
            )
            nc.sync.dma_start(out=out_own[:], in_=outt[:])

    nc.compile()
    return nc


_CACHE = {}


def kernel(x, edge_index, W0, b0, W1, b1, Wf, bf):
    x = np.asarray(x, dtype=np.float32)
    edge_index = np.asarray(edge_index)
    plan, totc, dinv, perms, offs_all = _build_plan_and_offsets(edge_index)

    key = ("prog", totc, len(plan))
    if key not in _CACHE:
        _CACHE[key] = _build_program(plan, totc)
    nc = _CACHE[key]

    in_maps = []
    rr = np.arange(NPC)
    pp_, gg = rr % P, rr // P
    for c in range(8):
        perm = perms[c]
        xo = np.zeros((P, G, F), dtype=np.float32)
        dv = np.zeros((P, G), dtype=np.float32)
        xo[pp_, gg, :] = x[perm]
        dv[pp_, gg] = dinv[perm]
        in_maps.append(
            {
                "x_own": xo.reshape(P, G * F),
                "dinv_own": dv,
                "offs": np.tile(offs_all[c], (8, 1)).astype(np.int16),
                "W0": np.asarray(W0, np.float32),
                "W1": np.asarray(W1, np.float32),
                "Wf": np.asarray(Wf, np.float32),
                "b0": np.asarray(b0, np.float32).reshape(F, 1),
                "b1": np.asarray(b1, np.float32).reshape(F, 1),
                "bf": np.asarray(bf, np.float32).reshape(F, 1),
            }
        )

    res = run_bass_kernel_spmd(nc, in_maps, list(range(8)))
    kernel._last_results = res

    out = np.zeros((N, F), dtype=np.float32)
    for c in range(8):
        oo = res.results[c]["out_own"].reshape(P, G, F)
        out[perms[c]] = oo[pp_, gg, :]
    return out


# revision 7
# speedup vs baseline: 1.9684x; 1.9684x over previous
"""Trainium2 Bass kernel for a 2-layer GCN (nn_CorrelationGNN).

Math (reference):
    src,dst = edges + self loops;  deg over dst;  dinv = deg^-1/2
    h1 = relu(S @ (x @ W0) + b0),  S = D^-1/2 (A+I) D^-1/2
    h2 = relu(S @ (h1 @ W1) + b1)
    out = h2 @ Wf + bf

Factorization used: S @ (h W) = dinv * Agg(dinv * h) @ W, where Agg is the
pure 0/1 adjacency gather-sum (S commutes with the feature matmul).

Distribution: destination nodes sharded across 8 cores (12500/core, padded
to 12544 = 128*98).  Ranks are degree-sorted; rank r -> (p=r%128, g=r//128),
table row within a core slice = p*98+g.  Gather source is an fp16 table
[100352, 128] (row = 32 feats + 96 zeros = 256B) assembled per core from an
AllGather of compact fp16 slices.  Edges are gathered with gpsimd dma_gather
(int16 idxs, 4 SWDGE queues, <=1024 idxs/inst) as 4 source-quarter streams;
per (quarter, g-column) the slot count K is the max over all cores so the
traced program is identical on every core (SPMD).
"""

import numpy as np

import concourse.bass as bass  # noqa: F401
import concourse.bacc as bacc
import concourse.mybir as mybir
from concourse.tile import TileContext
from concourse.bass_utils import run_bass_kernel_spmd

P = 128
N = 100000
F = 32
NPC = 12500          # real nodes per core
G = 98               # g-columns per core
NPCP = P * G         # padded nodes per core = 12544
NROWS = 8 * NPCP     # global table rows = 100352
QROWS = NROWS // 4   # 25088, int16-addressable quarter
QZREL = 12543        # guaranteed-zero pad row, same offset in every quarter
KCAP = 8             # slots per dma_gather inst (8*128 = 1024 idx cap)
FDT = mybir.dt.float32
HDT = mybir.dt.float16


def _build_plan_and_offsets(edge_index):
    src = np.asarray(edge_index[0], dtype=np.int64)
    dst = np.asarray(edge_index[1], dtype=np.int64)
    loops = np.arange(N, dtype=np.int64)
    src = np.concatenate([src, loops])
    dst = np.concatenate([dst, loops])

    deg = np.bincount(dst, minlength=N).astype(np.float64)
    dinv = (1.0 / np.sqrt(deg)).astype(np.float32)

    node_core = np.arange(N) // NPC
    rank = np.empty(N, dtype=np.int64)
    perms = []
    for c in range(8):
        nodes = np.arange(c * NPC, (c + 1) * NPC)
        order = np.argsort(-deg[nodes], kind="stable")
        perm = nodes[order]
        perms.append(perm)
        rank[perm] = np.arange(NPC)
    trow = node_core * NPCP + (rank % P) * G + (rank // P)
    quarter = trow // QROWS
    qrel = (trow % QROWS).astype(np.int32)

    # per-core edges sorted by (dst rank, src quarter)
    edges = []
    cnt_rq = np.zeros((8, NPC * 4), dtype=np.int32)
    for c in range(8):
        m = (dst // NPC) == c
        s, d = src[m], dst[m]
        key = rank[d] * 4 + quarter[s]
        order = np.argsort(key, kind="stable")
        edges.append((qrel[s][order], key[order]))
        cnt_rq[c] = np.bincount(key, minlength=NPC * 4)

    crq = cnt_rq.reshape(8, NPC, 4)
    K = np.zeros((G, 4), dtype=np.int32)
    for g in range(G):
        K[g] = crq[:, g * P : (g + 1) * P, :].max(axis=(0, 1))

    # shared instruction plan: (q, g, k0, kc, col0)
    plan = []
    col = 0
    for q in range(4):
        for g in range(G):
            k0 = 0
            while k0 < int(K[g, q]):
                kc = min(KCAP, int(K[g, q]) - k0)
                plan.append((q, g, k0, kc, col))
                col += kc * P // 16
                k0 += kc
    totc = col

    # zero pad rows (pad ranks 12500.. of the two cores in each quarter have
    # dinv=0 so their table rows are always zero); spread pad tokens across
    # them to avoid HBM hot-spotting on a single row.
    pad_ranks = np.arange(NPC, NPCP)
    zrel = (pad_ranks % P) * G + (pad_ranks // P)  # within-slice rows
    zero_rows = np.concatenate([zrel, zrel + NPCP]).astype(np.int16)  # both cores

    # per-core offset tables [16, totc], idx i of inst at [i%16, col0+i//16]
    offs_all = []
    for c in range(8):
        qr, key = edges[c]
        cnt = np.bincount(key, minlength=NPC * 4)
        ptr = np.zeros(NPC * 4 + 1, dtype=np.int64)
        np.cumsum(cnt, out=ptr[1:])
        rngpad = np.random.default_rng(c)
        offs = zero_rows[rngpad.integers(0, len(zero_rows), size=totc * 16)].astype(
            np.int16
        )
        for (q, g, k0, kc, col0) in plan:
            base = col0 * 16
            lo = g * P
            nreal = min(P, NPC - lo)
            # vectorized over p
            ps = np.arange(nreal)
            rk = lo + ps
            a = ptr[rk * 4 + q]
            b = ptr[rk * 4 + q + 1]
            for kk in range(kc):
                k = k0 + kk
                sel = (a + k) < b
                pos = base + kk * P + ps[sel]
                offs[pos] = qr[(a + k)[sel]]
        offs_all.append(offs.reshape(totc, 16).T.copy())

    return plan, totc, dinv, perms, offs_all


def _build_program(plan, totc):
    nc = bacc.Bacc(
        "TRN2", target_bir_lowering=False, debug=False, num_devices=8,
        num_swdge_queues=4,
    )
    x_own = nc.declare_dram_parameter("x_own", [P, G * F], FDT, isOutput=False)
    dinv_own = nc.declare_dram_parameter("dinv_own", [P, G], FDT, isOutput=False)
    offs = nc.declare_dram_parameter("offs", [P, totc], mybir.dt.int16, isOutput=False)
    W0 = nc.declare_dram_parameter("W0", [F, F], FDT, isOutput=False)
    W1 = nc.declare_dram_parameter("W1", [F, F], FDT, isOutput=False)
    Wf = nc.declare_dram_parameter("Wf", [F, F], FDT, isOutput=False)
    b0 = nc.declare_dram_parameter("b0", [F, 1], FDT, isOutput=False)
    b1 = nc.declare_dram_parameter("b1", [F, 1], FDT, isOutput=False)
    bf = nc.declare_dram_parameter("bf", [F, 1], FDT, isOutput=False)
    out_own = nc.declare_dram_parameter("out_own", [P, G * F], FDT, isOutput=True)

    cc_in = nc.dram_tensor("cc_in", [NPCP, F], HDT)
    cc_out = nc.dram_tensor("cc_out", [NROWS, F], HDT, addr_space="Shared")
    table = nc.dram_tensor("table", [NROWS, 4 * F], HDT)

    from concourse.masks import make_identity

    with TileContext(nc) as tc:
        with (
            tc.tile_pool(name="persist", bufs=1) as pp,
            tc.tile_pool(name="gpool", bufs=10) as gp,
            tc.tile_pool(name="spool", bufs=4) as sp,
            tc.tile_pool(name="psum", bufs=2, space="PSUM") as psp,
        ):
            offs_t = pp.tile([P, totc], mybir.dt.int16)
            nc.sync.dma_start(out=offs_t[:], in_=offs[:])
            dinv_t = pp.tile([P, G], FDT)
            nc.sync.dma_start(out=dinv_t[:], in_=dinv_own[:])
            w0_t = pp.tile([F, F], FDT)
            nc.sync.dma_start(out=w0_t[:], in_=W0[:])
            w1_t = pp.tile([F, F], FDT)
            nc.sync.dma_start(out=w1_t[:], in_=W1[:])
            wf_t = pp.tile([F, F], FDT)
            nc.sync.dma_start(out=wf_t[:], in_=Wf[:])
            b0_t = pp.tile([F, 1], FDT)
            nc.sync.dma_start(out=b0_t[:], in_=b0[:])
            b1_t = pp.tile([F, 1], FDT)
            nc.sync.dma_start(out=b1_t[:], in_=b1[:])
            bf_t = pp.tile([F, 1], FDT)
            nc.sync.dma_start(out=bf_t[:], in_=bf[:])
            ident = pp.tile([P, P], FDT)
            make_identity(nc, ident[:])

            xcur = pp.tile([P, G * F], FDT, tag="xcur")
            nc.sync.dma_start(out=xcur[:], in_=x_own[:])
            agg = pp.tile([P, G * F], FDT, tag="agg")
            xd_own = pp.tile([P, G * F], HDT, tag="xdown")

            dinv_b = dinv_t[:].to_broadcast([P, G, F])

            def scale_to_table(src_tile, scope):
                with nc.named_scope(scope):
                    nc.vector.tensor_tensor(
                        out=xd_own[:].rearrange("p (g f) -> p g f", f=F),
                        in0=src_tile[:].rearrange("p (g f) -> p g f", f=F),
                        in1=dinv_b,
                        op=mybir.AluOpType.mult,
                    )
                    nc.sync.dma_start(out=cc_in[:], in_=xd_own[:])
                    nc.gpsimd.collective_compute(
                        "AllGather",
                        mybir.AluOpType.bypass,
                        replica_groups=[list(range(8))],
                        ins=[cc_in[:]],
                        outs=[cc_out[:]],
                    )
                    for qq in range(4):
                        nc.sync.dma_start(
                            out=table[qq * QROWS : (qq + 1) * QROWS, :F],
                            in_=cc_out[qq * QROWS : (qq + 1) * QROWS, :],
                        )

            def gather_layer(scope):
                with nc.named_scope(scope):
                    nc.vector.memset(agg[:], 0.0)
                    for (q, g, k0, kc, col0) in plan:
                        gt = gp.tile([P, KCAP, 4 * F], HDT, tag="g")
                        nc.gpsimd.dma_gather(
                            out_ap=gt[:, :kc, :],
                            in_ap=table[q * QROWS : (q + 1) * QROWS, :],
                            idxs_ap=offs_t[:, col0 : col0 + kc * P // 16],
                            num_idxs=kc * P,
                            num_idxs_reg=kc * P,
                            elem_size=4 * F,
                            queue_num=(q * G + g) % 4,
                        )
                        red = sp.tile([P, 4, F], FDT, tag="red")
                        k = kc
                        if k > 4:
                            h = k - 4
                            nc.vector.tensor_add(
                                out=red[:, :h, :], in0=gt[:, 0:h, :F], in1=gt[:, 4 : 4 + h, :F]
                            )
                            if h < 4:
                                nc.vector.tensor_copy(out=red[:, h:4, :], in_=gt[:, h:4, :F])
                            k = 4
                        else:
                            nc.vector.tensor_copy(out=red[:, :k, :], in_=gt[:, :k, :F])
                        while k > 1:
                            h = k // 2
                            r = k - h
                            nc.vector.tensor_add(
                                out=red[:, :h, :], in0=red[:, :h, :], in1=red[:, r : r + h, :]
                            )
                            k = r
                        nc.vector.tensor_add(
                            out=agg[:, g * F : (g + 1) * F],
                            in0=agg[:, g * F : (g + 1) * F],
                            in1=red[:, 0, :],
                        )

            def layer_tail(W_t, bias_t, relu, dest, scope, W2_t=None, bias2_t=None):
                with nc.named_scope(scope):
                    nc.vector.tensor_tensor(
                        out=agg[:].rearrange("p (g f) -> p g f", f=F),
                        in0=agg[:].rearrange("p (g f) -> p g f", f=F),
                        in1=dinv_b,
                        op=mybir.AluOpType.mult,
                    )
                    for g in range(G):
                        ps1 = psp.tile([F, P], FDT, tag="ps1")
                        nc.tensor.matmul(
                            out=ps1[:], lhsT=agg[:, g * F : (g + 1) * F], rhs=ident[:],
                            start=True, stop=True,
                        )
                        s1 = sp.tile([F, P], FDT, tag="s1")
                        nc.vector.tensor_copy(out=s1[:], in_=ps1[:])
                        ps2 = psp.tile([F, P], FDT, tag="ps2")
                        nc.tensor.matmul(out=ps2[:], lhsT=W_t[:], rhs=s1[:], start=True, stop=True)
                        s2 = sp.tile([F, P], FDT, tag="s2")
                        if relu:
                            nc.scalar.activation(
                                out=s2[:], in_=ps2[:],
                                func=mybir.ActivationFunctionType.Relu,
                                bias=b0_t[:, :1] if bias_t is b0_t else bias_t[:, :1],
                                scale=1.0,
                            )
                        else:
                            nc.vector.tensor_scalar(
                                out=s2[:], in0=ps2[:], scalar1=bias_t[:, :1],
                                scalar2=None, op0=mybir.AluOpType.add,
                            )
                        if W2_t is not None:
                            ps3 = psp.tile([F, P], FDT, tag="ps3")
                            nc.tensor.matmul(out=ps3[:], lhsT=W2_t[:], rhs=s2[:], start=True, stop=True)
                            s2b = sp.tile([F, P], FDT, tag="s2b")
                            nc.vector.tensor_scalar(
                                out=s2b[:], in0=ps3[:], scalar1=bias2_t[:, :1],
                                scalar2=None, op0=mybir.AluOpType.add,
                            )
                            s2 = s2b
                        psb = psp.tile([P, F], FDT, tag="psb")
                        nc.tensor.matmul(
                            out=psb[:], lhsT=s2[:], rhs=ident[:F, :F], start=True, stop=True
                        )
                        nc.vector.tensor_copy(out=dest[:, g * F : (g + 1) * F], in_=psb[:])

            scale_to_table(xcur, "table0")
            gather_layer("gather0")
            layer_tail(w0_t, b0_t, relu=True, dest=xcur, scope="tail0")
            scale_to_table(xcur, "table1")
            gather_layer("gather1")
            outt = pp.tile([P, G * F], FDT, tag="outt")
            layer_tail(
                w1_t, b1_t, relu=True, dest=outt, scope="tail1", W2_t=wf_t, bias2_t=bf_t
            )
            nc.sync.dma_start(out=out_own[:], in_=outt[:])

    nc.compile()
    return nc


_CACHE = {}


def kernel(x, edge_index, W0, b0, W1, b1, Wf, bf):
    x = np.asarray(x, dtype=np.float32)
    edge_index = np.asarray(edge_index)
    plan, totc, dinv, perms, offs_all = _build_plan_and_offsets(edge_index)

    key = ("prog", totc, len(plan))
    if key not in _CACHE:
        _CACHE[key] = _build_program(plan, totc)
    nc = _CACHE[key]

    in_maps = []
    rr = np.arange(NPC)
    pp_, gg = rr % P, rr // P
    for c in range(8):
        perm = perms[c]
        xo = np.zeros((P, G, F), dtype=np.float32)
        dv = np.zeros((P, G), dtype=np.float32)
        xo[pp_, gg, :] = x[perm]
        dv[pp_, gg] = dinv[perm]
        in_maps.append(
            {
                "x_own": xo.reshape(P, G * F),
                "dinv_own": dv,
                "offs": np.tile(offs_all[c], (8, 1)).astype(np.int16),
                "W0": np.asarray(W0, np.float32),
                "W1": np.asarray(W1, np.float32),
                "Wf": np.asarray(Wf, np.float32),
                "b0": np.asarray(b0, np.float32).reshape(F, 1),
                "b1": np.asarray(b1, np.float32).reshape(F, 1),
                "bf": np.asarray(bf, np.float32).reshape(F, 1),
            }
        )

    res = run_bass_kernel_spmd(nc, in_maps, list(range(8)))
    kernel._last_results = res

    out = np.zeros((N, F), dtype=np.float32)
    for c in range(8):
        oo = res.results[c]["out_own"].reshape(P, G, F)
        out[perms[c]] = oo[pp_, gg, :]
    return out


# revision 8
# speedup vs baseline: 2.8911x; 1.4688x over previous
"""Trainium2 Bass kernel for a 2-layer GCN (nn_CorrelationGNN).

Math (reference):
    src,dst = edges + self loops;  deg over dst;  dinv = deg^-1/2
    h1 = relu(S @ (x @ W0) + b0),  S = D^-1/2 (A+I) D^-1/2
    h2 = relu(S @ (h1 @ W1) + b1)
    out = h2 @ Wf + bf

Factorization used: S @ (h W) = dinv * Agg(dinv * h) @ W, where Agg is the
pure 0/1 adjacency gather-sum (S commutes with the feature matmul).

Distribution: destination nodes sharded across 8 cores (12500/core, padded
to 12544 = 128*98).  Ranks are degree-sorted; rank r -> (p=r%128, g=r//128),
table row within a core slice = p*98+g.  Gather source is an fp16 table
[100352, 128] (row = 32 feats + 96 zeros = 256B) assembled per core from an
AllGather of compact fp16 slices.  Edges are gathered with gpsimd dma_gather
(int16 idxs, 4 SWDGE queues, <=1024 idxs/inst) as 4 source-quarter streams;
per (quarter, g-column) the slot count K is the max over all cores so the
traced program is identical on every core (SPMD).
"""

import numpy as np

import concourse.bass as bass  # noqa: F401
import concourse.bacc as bacc
import concourse.mybir as mybir
from concourse.tile import TileContext
from concourse.bass_utils import run_bass_kernel_spmd

P = 128
N = 100000
F = 32
NPC = 12500          # real nodes per core
G = 98               # g-columns per core
NPCP = P * G         # padded nodes per core = 12544
NROWS = 8 * NPCP     # global table rows = 100352
QROWS = NROWS // 4   # 25088, int16-addressable quarter
QZREL = 12543        # guaranteed-zero pad row, same offset in every quarter
KCAP = 8             # slots per dma_gather inst (8*128 = 1024 idx cap)
FDT = mybir.dt.float32
HDT = mybir.dt.float16


def _build_plan_and_offsets(edge_index):
    src = np.asarray(edge_index[0], dtype=np.int64)
    dst = np.asarray(edge_index[1], dtype=np.int64)
    loops = np.arange(N, dtype=np.int64)
    src = np.concatenate([src, loops])
    dst = np.concatenate([dst, loops])

    deg = np.bincount(dst, minlength=N).astype(np.float64)
    dinv = (1.0 / np.sqrt(deg)).astype(np.float32)

    node_core = np.arange(N) // NPC
    rank = np.empty(N, dtype=np.int64)
    perms = []
    for c in range(8):
        nodes = np.arange(c * NPC, (c + 1) * NPC)
        order = np.argsort(-deg[nodes], kind="stable")
        perm = nodes[order]
        perms.append(perm)
        rank[perm] = np.arange(NPC)
    trow = node_core * NPCP + (rank % P) * G + (rank // P)
    quarter = trow // QROWS
    qrel = (trow % QROWS).astype(np.int32)

    # per-core edges sorted by (dst rank, src quarter); self-loops handled
    # on-device as agg init = xd_own, so drop them from the token streams
    noloop = src != dst
    srcn, dstn = src[noloop], dst[noloop]
    edges = []
    cnt_rq = np.zeros((8, NPC * 4), dtype=np.int32)
    for c in range(8):
        m = (dstn // NPC) == c
        s, d = srcn[m], dstn[m]
        key = rank[d] * 4 + quarter[s]
        order = np.argsort(key, kind="stable")
        edges.append((qrel[s][order], key[order]))
        cnt_rq[c] = np.bincount(key, minlength=NPC * 4)

    crq = cnt_rq.reshape(8, NPC, 4)
    K = np.zeros((G, 4), dtype=np.int32)
    for g in range(G):
        K[g] = crq[:, g * P : (g + 1) * P, :].max(axis=(0, 1))

    # shared instruction plan: (q, g, k0, kc, col0)
    plan = []
    col = 0
    for q in range(4):
        for g in range(G):
            k0 = 0
            while k0 < int(K[g, q]):
                kc = min(KCAP, int(K[g, q]) - k0)
                plan.append((q, g, k0, kc, col))
                col += kc * P // 16
                k0 += kc
    totc = col

    # zero pad rows (pad ranks 12500.. of the two cores in each quarter have
    # dinv=0 so their table rows are always zero); spread pad tokens across
    # them to avoid HBM hot-spotting on a single row.
    pad_ranks = np.arange(NPC, NPCP)
    zrel = (pad_ranks % P) * G + (pad_ranks // P)  # within-slice rows
    zero_rows = np.concatenate([zrel, zrel + NPCP]).astype(np.int16)  # both cores

    # per-core offset tables [16, totc], idx i of inst at [i%16, col0+i//16]
    offs_all = []
    for c in range(8):
        qr, key = edges[c]
        cnt = np.bincount(key, minlength=NPC * 4)
        ptr = np.zeros(NPC * 4 + 1, dtype=np.int64)
        np.cumsum(cnt, out=ptr[1:])
        rngpad = np.random.default_rng(c)
        offs = zero_rows[rngpad.integers(0, len(zero_rows), size=totc * 16)].astype(
            np.int16
        )
        for (q, g, k0, kc, col0) in plan:
            base = col0 * 16
            lo = g * P
            nreal = min(P, NPC - lo)
            # vectorized over p
            ps = np.arange(nreal)
            rk = lo + ps
            a = ptr[rk * 4 + q]
            b = ptr[rk * 4 + q + 1]
            for kk in range(kc):
                k = k0 + kk
                sel = (a + k) < b
                pos = base + kk * P + ps[sel]
                offs[pos] = qr[(a + k)[sel]]
        offs_all.append(offs.reshape(totc, 16).T.copy())

    return plan, totc, dinv, perms, offs_all


def _build_program(plan, totc):
    nc = bacc.Bacc(
        "TRN2", target_bir_lowering=False, debug=False, num_devices=8,
        num_swdge_queues=4,
    )
    x_own = nc.declare_dram_parameter("x_own", [P, G * F], FDT, isOutput=False)
    dinv_own = nc.declare_dram_parameter("dinv_own", [P, G], FDT, isOutput=False)
    offs = nc.declare_dram_parameter("offs", [P, totc], mybir.dt.int16, isOutput=False)
    W0 = nc.declare_dram_parameter("W0", [F, F], FDT, isOutput=False)
    W1 = nc.declare_dram_parameter("W1", [F, F], FDT, isOutput=False)
    Wf = nc.declare_dram_parameter("Wf", [F, F], FDT, isOutput=False)
    b0 = nc.declare_dram_parameter("b0", [F, 1], FDT, isOutput=False)
    b1 = nc.declare_dram_parameter("b1", [F, 1], FDT, isOutput=False)
    bf = nc.declare_dram_parameter("bf", [F, 1], FDT, isOutput=False)
    out_own = nc.declare_dram_parameter("out_own", [P, G * F], FDT, isOutput=True)

    cc_in = nc.dram_tensor("cc_in", [NPCP, F], HDT)
    cc_out = nc.dram_tensor("cc_out", [NROWS, F], HDT, addr_space="Shared")
    table = nc.dram_tensor("table", [NROWS, 4 * F], HDT)

    from concourse.masks import make_identity

    with TileContext(nc) as tc:
        with (
            tc.tile_pool(name="persist", bufs=1) as pp,
            tc.tile_pool(name="gpool", bufs=10) as gp,
            tc.tile_pool(name="spool", bufs=4) as sp,
            tc.tile_pool(name="psum", bufs=2, space="PSUM") as psp,
        ):
            offs_t = pp.tile([P, totc], mybir.dt.int16)
            nc.sync.dma_start(out=offs_t[:], in_=offs[:])
            dinv_t = pp.tile([P, G], FDT)
            nc.sync.dma_start(out=dinv_t[:], in_=dinv_own[:])
            w0_t = pp.tile([F, F], FDT)
            nc.sync.dma_start(out=w0_t[:], in_=W0[:])
            w1_t = pp.tile([F, F], FDT)
            nc.sync.dma_start(out=w1_t[:], in_=W1[:])
            wf_t = pp.tile([F, F], FDT)
            nc.sync.dma_start(out=wf_t[:], in_=Wf[:])
            b0_t = pp.tile([F, 1], FDT)
            nc.sync.dma_start(out=b0_t[:], in_=b0[:])
            b1_t = pp.tile([F, 1], FDT)
            nc.sync.dma_start(out=b1_t[:], in_=b1[:])
            bf_t = pp.tile([F, 1], FDT)
            nc.sync.dma_start(out=bf_t[:], in_=bf[:])
            ident = pp.tile([P, P], FDT)
            make_identity(nc, ident[:])

            xcur = pp.tile([P, G * F], FDT, tag="xcur")
            nc.sync.dma_start(out=xcur[:], in_=x_own[:])
            agg = pp.tile([P, G * F], FDT, tag="agg")
            xd_own = pp.tile([P, G * F], HDT, tag="xdown")

            dinv_b = dinv_t[:].to_broadcast([P, G, F])

            def scale_to_table(src_tile, scope):
                with nc.named_scope(scope):
                    nc.vector.tensor_tensor(
                        out=xd_own[:].rearrange("p (g f) -> p g f", f=F),
                        in0=src_tile[:].rearrange("p (g f) -> p g f", f=F),
                        in1=dinv_b,
                        op=mybir.AluOpType.mult,
                    )
                    nc.sync.dma_start(out=cc_in[:], in_=xd_own[:])
                    nc.gpsimd.collective_compute(
                        "AllGather",
                        mybir.AluOpType.bypass,
                        replica_groups=[list(range(8))],
                        ins=[cc_in[:]],
                        outs=[cc_out[:]],
                    )
                    for qq in range(4):
                        nc.sync.dma_start(
                            out=table[qq * QROWS : (qq + 1) * QROWS, :F],
                            in_=cc_out[qq * QROWS : (qq + 1) * QROWS, :],
                        )

            def gather_layer(scope):
                with nc.named_scope(scope):
                    # self-loop contribution: agg starts at xd_own
                    nc.vector.tensor_copy(out=agg[:], in_=xd_own[:])
                    for (q, g, k0, kc, col0) in plan:
                        gt = gp.tile([P, KCAP, 4 * F], HDT, tag="g")
                        nc.gpsimd.dma_gather(
                            out_ap=gt[:, :kc, :],
                            in_ap=table[q * QROWS : (q + 1) * QROWS, :],
                            idxs_ap=offs_t[:, col0 : col0 + kc * P // 16],
                            num_idxs=kc * P,
                            num_idxs_reg=kc * P,
                            elem_size=4 * F,
                            queue_num=(q * G + g) % 4,
                        )
                        if kc == 1:
                            nc.vector.tensor_add(
                                out=agg[:, g * F : (g + 1) * F],
                                in0=agg[:, g * F : (g + 1) * F],
                                in1=gt[:, 0, :F],
                            )
                        else:
                            red = sp.tile([P, F], FDT, tag="red")
                            nc.vector.reduce_sum(
                                out=red[:],
                                in_=gt[:, :kc, :F].rearrange("p k f -> p f k"),
                                axis=mybir.AxisListType.X,
                            )
                            nc.vector.tensor_add(
                                out=agg[:, g * F : (g + 1) * F],
                                in0=agg[:, g * F : (g + 1) * F],
                                in1=red[:],
                            )

            def layer_tail(W_t, bias_t, relu, dest, scope, W2_t=None, bias2_t=None):
                with nc.named_scope(scope):
                    nc.vector.tensor_tensor(
                        out=agg[:].rearrange("p (g f) -> p g f", f=F),
                        in0=agg[:].rearrange("p (g f) -> p g f", f=F),
                        in1=dinv_b,
                        op=mybir.AluOpType.mult,
                    )
                    for g in range(G):
                        ps1 = psp.tile([F, P], FDT, tag="ps1")
                        nc.tensor.matmul(
                            out=ps1[:], lhsT=agg[:, g * F : (g + 1) * F], rhs=ident[:],
                            start=True, stop=True,
                        )
                        s1 = sp.tile([F, P], FDT, tag="s1")
                        nc.vector.tensor_copy(out=s1[:], in_=ps1[:])
                        ps2 = psp.tile([F, P], FDT, tag="ps2")
                        nc.tensor.matmul(out=ps2[:], lhsT=W_t[:], rhs=s1[:], start=True, stop=True)
                        s2 = sp.tile([F, P], FDT, tag="s2")
                        if relu:
                            nc.scalar.activation(
                                out=s2[:], in_=ps2[:],
                                func=mybir.ActivationFunctionType.Relu,
                                bias=b0_t[:, :1] if bias_t is b0_t else bias_t[:, :1],
                                scale=1.0,
                            )
                        else:
                            nc.vector.tensor_scalar(
                                out=s2[:], in0=ps2[:], scalar1=bias_t[:, :1],
                                scalar2=None, op0=mybir.AluOpType.add,
                            )
                        if W2_t is not None:
                            ps3 = psp.tile([F, P], FDT, tag="ps3")
                            nc.tensor.matmul(out=ps3[:], lhsT=W2_t[:], rhs=s2[:], start=True, stop=True)
                            s2b = sp.tile([F, P], FDT, tag="s2b")
                            nc.vector.tensor_scalar(
                                out=s2b[:], in0=ps3[:], scalar1=bias2_t[:, :1],
                                scalar2=None, op0=mybir.AluOpType.add,
                            )
                            s2 = s2b
                        psb = psp.tile([P, F], FDT, tag="psb")
                        nc.tensor.matmul(
                            out=psb[:], lhsT=s2[:], rhs=ident[:F, :F], start=True, stop=True
                        )
                        nc.vector.tensor_copy(out=dest[:, g * F : (g + 1) * F], in_=psb[:])

            scale_to_table(xcur, "table0")
            gather_layer("gather0")
            layer_tail(w0_t, b0_t, relu=True, dest=xcur, scope="tail0")
            scale_to_table(xcur, "table1")
            gather_layer("gather1")
            outt = pp.tile([P, G * F], FDT, tag="outt")
            layer_tail(
                w1_t, b1_t, relu=True, dest=outt, scope="tail1", W2_t=wf_t, bias2_t=bf_t
            )
            nc.sync.dma_start(out=out_own[:], in_=outt[:])

    nc.compile()
    return nc


_CACHE = {}


def kernel(x, edge_index, W0, b0, W1, b1, Wf, bf):
    x = np.asarray(x, dtype=np.float32)
    edge_index = np.asarray(edge_index)
    plan, totc, dinv, perms, offs_all = _build_plan_and_offsets(edge_index)

    key = ("prog", totc, len(plan))
    if key not in _CACHE:
        _CACHE[key] = _build_program(plan, totc)
    nc = _CACHE[key]

    in_maps = []
    rr = np.arange(NPC)
    pp_, gg = rr % P, rr // P
    for c in range(8):
        perm = perms[c]
        xo = np.zeros((P, G, F), dtype=np.float32)
        dv = np.zeros((P, G), dtype=np.float32)
        xo[pp_, gg, :] = x[perm]
        dv[pp_, gg] = dinv[perm]
        in_maps.append(
            {
                "x_own": xo.reshape(P, G * F),
                "dinv_own": dv,
                "offs": np.tile(offs_all[c], (8, 1)).astype(np.int16),
                "W0": np.asarray(W0, np.float32),
                "W1": np.asarray(W1, np.float32),
                "Wf": np.asarray(Wf, np.float32),
                "b0": np.asarray(b0, np.float32).reshape(F, 1),
                "b1": np.asarray(b1, np.float32).reshape(F, 1),
                "bf": np.asarray(bf, np.float32).reshape(F, 1),
            }
        )

    res = run_bass_kernel_spmd(nc, in_maps, list(range(8)))
    kernel._last_results = res

    out = np.zeros((N, F), dtype=np.float32)
    for c in range(8):
        oo = res.results[c]["out_own"].reshape(P, G, F)
        out[perms[c]] = oo[pp_, gg, :]
    return out
